# revision 47
# baseline (speedup 1.0000x reference)
"""Trainium2 Bass kernel for a LLaMA-style causal attention block.

Sharding (8 NeuronCores, one trn2 chip):
  - Tensor-parallel over heads: core c owns heads [4c, 4c+4) -> wq/wk/wv column
    slices [4096, 512]; computes qT/kT/v + RoPE + causal attention for its heads.
  - attnT [512, 2048] (bf16) is AllGather'd per sq quarter in two head-pair
    chunks (issued mid-attention so comm overlaps compute) -> each core computes
    out[:, 512c:512c+512] = attn @ wo_cols, interleaved into the next quarter's
    attention so the PE never drains.
  - Host concatenates the 8 column slices.

Layout trick: everything is computed transposed ([head_dim, seq]) so that no
on-device transposes are needed anywhere:
  qT/kT = w_h.T @ xT      (xT host-pretransposed)
  scoresT[sk, sq] = kT_tile.T @ qT     (softmax denom off the PE critical path)
  attnT[hd, sq] = v_tile.T @ expT      (expT is exactly the scoresT layout)
  out[sq, cols] = attnT_full_tile.T @ wo_tile
RoPE is applied in the transposed layout with a DVE stream_shuffle partition
pair-swap. exp() needs no max-subtraction: scores are O(1) by construction.

Softmax denominator: exp tiles are accumulated on the DVE into an f32 SBUF
tile; one [128x128] all-ones matmul per (head, quarter) produces the
column-sums pre-broadcast across partitions (recip on DVE, no GpSimd).

x tile loads for round st+1 are emitted at the end of round st so the DMA
queues drain them during the attention phase instead of stalling the next
round's first matmuls.

Compute dtype bf16 (f32 PSUM accumulation), I/O f32.
"""

import math
import os
import sys

for _p in ("/opt/trn_rl_repo",):
    if os.path.isdir(_p) and _p not in sys.path:
        sys.path.insert(0, _p)

import numpy as np
import ml_dtypes

N_CORES = 8
B, S, D, H = 1, 2048, 4096, 32
HD = D // H          # 128
HPC = H // N_CORES   # 4 heads per core
CW = D // N_CORES    # 512 columns per core
NK = D // 128        # 32 contraction tiles
SQT = 512            # sq tile width
NSQ = S // SQT       # 4
SCALE = 1.0 / math.sqrt(HD)
PVLAG = 2            # sk-blocks of lead the exp pipeline gets over PV

_CACHE = {}
LAST_RESULT = None   # test harness reads exec_time_ns from here


def _build():
    import concourse.mybir as mybir
    import concourse.tile as tile
    from concourse import bacc, bass_isa

    dt = mybir.dt
    f32, bf16 = dt.float32, dt.bfloat16

    nc = bacc.Bacc("TRN2", target_bir_lowering=False, debug=False,
                   num_devices=N_CORES)

    xT = nc.dram_tensor("xT", [D, S], bf16, kind="ExternalInput").ap()
    wq = nc.dram_tensor("wq", [D, CW], bf16, kind="ExternalInput").ap()
    wk = nc.dram_tensor("wk", [D, CW], bf16, kind="ExternalInput").ap()
    wv = nc.dram_tensor("wv", [D, CW], bf16, kind="ExternalInput").ap()
    wo = nc.dram_tensor("wo", [D, CW], bf16, kind="ExternalInput").ap()
    cosT = nc.dram_tensor("cosT", [HD, S], bf16, kind="ExternalInput").ap()
    sinT = nc.dram_tensor("sinT", [HD, S], bf16, kind="ExternalInput").ap()
    ones = nc.dram_tensor("ones", [128, 128], bf16, kind="ExternalInput").ap()
    masks = nc.dram_tensor("masks", [4, 128, SQT], bf16, kind="ExternalInput").ap()
    out = nc.dram_tensor("out", [S, CW], f32, kind="ExternalOutput").ap()

    swap_mask = []
    for i in range(16):
        swap_mask += [2 * i + 1, 2 * i]

    rg = [list(range(N_CORES))]

    with tile.TileContext(nc) as tc:
        with (
            tc.tile_pool(name="consts", bufs=1) as cpool,
            tc.tile_pool(name="xp", bufs=34) as xpool,
            tc.tile_pool(name="wqp", bufs=10) as wqp,
            tc.tile_pool(name="wkp", bufs=10) as wkp,
            tc.tile_pool(name="wvp", bufs=11) as wvp,
            tc.tile_pool(name="res", bufs=1) as res,
            tc.tile_pool(name="qro", bufs=8) as qro,
            tc.tile_pool(name="rope32", bufs=4) as rope32,
            tc.tile_pool(name="ropebf", bufs=3) as ropebf,
            tc.tile_pool(name="expp", bufs=9) as expp,
            tc.tile_pool(name="accp", bufs=2) as accp,
            tc.tile_pool(name="aevp", bufs=2) as aevp,
            tc.tile_pool(name="nrm", bufs=2) as nrm,
            tc.tile_pool(name="attnsb", bufs=3) as attnsb,
            tc.tile_pool(name="wop", bufs=1) as wop,
            tc.tile_pool(name="agsb", bufs=6) as agsb,
            tc.tile_pool(name="osb", bufs=4) as osb,
            tc.tile_pool(name="ps", bufs=8, space="PSUM") as ps,
            tc.tile_pool(name="dram", bufs=1, space="DRAM") as dram,
        ):
            # resident results of K+rope and V; Q lives in a rotating pool
            krot = [res.tile([HD, S], bf16, name=f"krot{h}") for h in range(HPC)]
            v_sb = [res.tile([128, CW], bf16, name=f"v{i}") for i in range(S // 128)]

            # AllGather bounce buffers: per quarter, two head chunks (A, B);
            # quarter 3 is split 3+1 so the tail projection covers B's latency
            AG_SPLIT = [2, 2, 2, 2]
            ag_in = [[dram.tile([a * HD, SQT], bf16, name=f"agin{q}_{c}")
                      for c, a in enumerate((AG_SPLIT[q], HPC - AG_SPLIT[q]))]
                     for q in range(NSQ)]
            ag_out = [[dram.tile([N_CORES * a * HD, SQT], bf16,
                                 addr_space="Shared", name=f"agout{q}_{c}")
                       for c, a in enumerate((AG_SPLIT[q], HPC - AG_SPLIT[q]))]
                      for q in range(NSQ)]

            cos_sb = cpool.tile([HD, S], bf16, name="cos_sb")
            ones_sb = cpool.tile([128, 128], bf16, name="ones_sb")
            sin_sb = cpool.tile([HD, S], bf16, name="sin_sb")
            mask_sb = [cpool.tile([128, SQT], bf16, name=f"mask{r}")
                       for r in range(4)]
            wo_sb = [wop.tile([128, CW], bf16, name=f"wo{d}") for d in range(NK)]

            WPRE = 10  # weight tiles of the next round pre-emitted per proj
            WPRE0 = NK  # round 0 is fully cold: interleave every weight pair

            def emit_x_loads(st):
                # first 8 wq/wk tiles of round st go in FRONT of the x bulk so
                # the round's first matmuls never wait behind 4 MB of x
                sq0 = st * SQT
                pre_w = []
                tiles = []
                npre = WPRE0 if st == 0 else WPRE
                for d in range(NK):
                    if d < npre:
                        wqt = wqp.tile([128, CW], bf16, tag="wq",
                                       name=f"wq{st}_{d}")
                        nc.sync.dma_start(wqt[:], wq[d * 128:(d + 1) * 128, :])
                        wkt = wkp.tile([128, CW], bf16, tag="wk",
                                       name=f"wk{st}_{d}")
                        nc.sync.dma_start(wkt[:], wk[d * 128:(d + 1) * 128, :])
                        pre_w.append((wqt, wkt))
                    xt = xpool.tile([128, SQT], bf16, tag="x", name=f"x{st}_{d}")
                    nc.sync.dma_start(xt[:], xT[d * 128:(d + 1) * 128,
                                                sq0:sq0 + SQT])
                    tiles.append(xt)
                return tiles, pre_w

            qrot_tiles = {}  # (st, h) -> tile

            def emit_qkv(st, x_tiles, pre_w):
                sq0 = st * SQT
                q_ps = [ps.tile([128, SQT], f32, tag="b", name=f"qps{st}_{h}")
                        for h in range(HPC)]
                k_ps = [ps.tile([128, SQT], f32, tag="b", name=f"kps{st}_{h}")
                        for h in range(HPC)]
                for d in range(NK):
                    xt = x_tiles[d]
                    if d < len(pre_w):
                        wqt, wkt = pre_w[d]
                    else:
                        wqt = wqp.tile([128, CW], bf16, tag="wq",
                                       name=f"wq{st}_{d}")
                        nc.sync.dma_start(wqt[:], wq[d * 128:(d + 1) * 128, :])
                        wkt = wkp.tile([128, CW], bf16, tag="wk",
                                       name=f"wk{st}_{d}")
                        nc.sync.dma_start(wkt[:], wk[d * 128:(d + 1) * 128, :])
                    first, last = d == 0, d == NK - 1
                    for h in range(HPC):
                        nc.tensor.matmul(q_ps[h][:], wqt[:, h * HD:(h + 1) * HD],
                                         xt[:], start=first, stop=last)
                    for h in range(HPC):
                        nc.tensor.matmul(k_ps[h][:], wkt[:, h * HD:(h + 1) * HD],
                                         xt[:], start=first, stop=last)
                if st == 0:
                    # constants are first needed by RoPE / attention below;
                    # emitting them here keeps the first QKV DMAs in front
                    nc.sync.dma_start(cos_sb[:], cosT[:])
                    nc.sync.dma_start(sin_sb[:], sinT[:])
                    nc.sync.dma_start(ones_sb[:], ones[:])
                    for r in range(4):
                        nc.sync.dma_start(mask_sb[r][:], masks[r])
                # RoPE: rot = t*cos + shuffle(t)*sin'   (sin' sign-baked)
                for h in range(HPC):
                    qt = qro.tile([HD, SQT], bf16, tag="q", name=f"qrot{st}_{h}")
                    qrot_tiles[(st, h)] = qt
                    for pst, rot in ((q_ps[h], qt[:]),
                                     (k_ps[h], krot[h][:, sq0:sq0 + SQT])):
                        tbf = ropebf.tile([128, SQT], bf16, tag="rbf",
                                          name=f"rbf{st}_{h}")
                        nc.scalar.copy(tbf[:], pst[:])
                        tsw = ropebf.tile([128, SQT], bf16, tag="rsw",
                                          name=f"rsw{st}_{h}")
                        nc.vector.stream_shuffle(tsw[:], tbf[:], swap_mask)
                        t1 = rope32.tile([128, SQT], f32, tag="r32",
                                         name=f"r1_{st}_{h}")
                        nc.vector.tensor_mul(t1[:], tbf[:],
                                             cos_sb[:, sq0:sq0 + SQT])
                        t2 = rope32.tile([128, SQT], f32, tag="r32",
                                         name=f"r2_{st}_{h}")
                        nc.vector.tensor_mul(t2[:], tsw[:],
                                             sin_sb[:, sq0:sq0 + SQT])
                        nc.vector.tensor_add(rot, t1[:], t2[:])
                # V projection for this s range; all wv loads are issued
                # up front so the first V matmuls never wait on DMA
                wv_tiles = []
                for d in range(NK):
                    wvt = wvp.tile([128, CW], bf16, tag="wv", name=f"wv{st}_{d}")
                    nc.sync.dma_start(wvt[:], wv[d * 128:(d + 1) * 128, :])
                    wv_tiles.append(wvt)
                v_ps = [ps.tile([128, CW], f32, tag="b", name=f"vps{st}_{ss}")
                        for ss in range(4)]
                for d in range(NK):
                    first, last = d == 0, d == NK - 1
                    for ss in range(4):
                        nc.tensor.matmul(v_ps[ss][:],
                                         x_tiles[d][:, ss * 128:(ss + 1) * 128],
                                         wv_tiles[d][:], start=first, stop=last)
                for ss in range(4):
                    nc.scalar.copy(v_sb[st * 4 + ss][:], v_ps[ss][:])

            # ---------- output projection, interleaved per sk-block ----------
            pending_out = {}  # q -> o_tiles awaiting store

            class OProj:
                """Emits the output projection of quarter q in per-d slices so
                it can be spread across the next quarter's attention blocks,
                filling PE pipeline bubbles without spiking PSUM pressure."""

                def __init__(self, q, a_first=False, defer_b=False):
                    self.q = q
                    self.d = 0
                    # a_first: consume the A head-chunk's d-tiles first so the
                    # tail projection's matmuls cover the B AllGather.
                    # defer_b: don't emit B-chunk loads until finish() — used
                    # when construction precedes the B AllGather's emission
                    # (loads emitted before their writer would read garbage).
                    sp = AG_SPLIT[q]
                    if a_first:
                        self.order = ([d for d in range(NK) if d % 4 < sp]
                                      + [d for d in range(NK) if d % 4 >= sp])
                        self.na = sum(1 for d in range(NK) if d % 4 < sp)
                    else:
                        self.order = list(range(NK))
                        self.na = NK
                    self.defer_b = defer_b
                    self.agt = {}
                    self.o_ps = [ps.tile([128, CW], f32, tag="b",
                                         name=f"ops{q}_{ss}")
                                 for ss in range(4)]
                    self.loaded = 0
                    for i in range(4 if a_first else 2):
                        self._load(i)
                        self.loaded += 1

                def _load(self, i):
                    if i >= NK:
                        return
                    d = self.order[i]
                    c, lh = d // 4, d % 4
                    sp = AG_SPLIT[self.q]
                    if lh < sp:
                        src, row = ag_out[self.q][0], c * sp * HD + lh * HD
                    else:
                        src = ag_out[self.q][1]
                        row = c * (HPC - sp) * HD + (lh - sp) * HD
                    agt = agsb.tile([128, SQT], bf16, tag="ag",
                                    name=f"agt{self.q}_{d}")
                    nc.sync.dma_start(agt[:], src[row:row + HD, :])
                    self.agt[d] = agt

                def step(self):
                    if self.d >= NK:
                        return
                    i = self.d
                    d = self.order[i]
                    cap = self.na if self.defer_b else NK
                    while self.loaded < min(cap, i + 3):
                        self._load(self.loaded)
                        self.loaded += 1
                    first, last = i == 0, i == NK - 1
                    for ss in range(4):
                        nc.tensor.matmul(self.o_ps[ss][:],
                                         self.agt[d][:, ss * 128:(ss + 1) * 128],
                                         wo_sb[d][:], start=first, stop=last)
                    self.d += 1

                def finish(self):
                    self.defer_b = False
                    while self.d < NK:
                        self.step()
                    o_tiles = []
                    for ss in range(4):
                        o = osb.tile([128, CW], f32, tag="o",
                                     name=f"o{self.q}_{ss}")
                        nc.scalar.copy(o[:], self.o_ps[ss][:])
                        o_tiles.append(o)
                    pending_out[self.q] = o_tiles

            def emit_out_stores(q):
                tiles = pending_out.pop(q)
                for ss in range(4):
                    nc.sync.dma_start(
                        out[q * SQT + ss * 128:q * SQT + (ss + 1) * 128, :],
                        tiles[ss][:])

            def emit_attention(sqT, oq, oq_next=None):
                # attention for quarter sqT; if oq is not None, interleave the
                # output projection of quarter oq between sk-blocks.  With
                # oq_next set (last quarter), oq is packed into the first half
                # and oq_next's A-chunk tiles into the second half so only its
                # B-chunk remains after the final AllGather.
                nblk = 4 * (sqT + 1)
                op = OProj(oq) if oq is not None else None
                op2 = None
                nblk_total = nblk * HPC
                half = nblk_total // 2
                na = NK // 2  # A-chunk d-tiles in a_first order
                blocks_done = 0
                for h in range(HPC):
                    attn_ps = ps.tile([HD, SQT], f32, tag="b",
                                      name=f"aps{sqT}_{h}")
                    eacc = accp.tile([128, SQT], bf16, tag="acc",
                                     name=f"acc{sqT}_{h}")
                    exp_tiles = []

                    def emit_pv(j, h=h, attn_ps=attn_ps,
                                exp_tiles=exp_tiles, nblk=nblk):
                        first, last = j == 0, j == nblk - 1
                        e, off = exp_tiles[j]
                        n = SQT - off
                        nc.tensor.matmul(attn_ps[:, off:SQT],
                                         v_sb[j][:, h * HD:(h + 1) * HD],
                                         e[:, 0:n],
                                         start=first, stop=last)

                    for i in range(nblk):
                        r = i - 4 * sqT
                        # diagonal blocks: only sq >= sk is valid; skip the
                        # fully-masked leading columns entirely
                        off = max(0, r) * 128
                        n = SQT - off
                        sc = ps.tile([128, SQT], f32, tag="b",
                                     name=f"sc{sqT}_{h}_{i}")
                        nc.tensor.matmul(sc[:, 0:n],
                                         krot[h][:, i * 128:(i + 1) * 128],
                                         qrot_tiles[(sqT, h)][:, off:SQT],
                                         start=True, stop=True)
                        if r >= 0:  # triangular part within the first strip
                            nc.vector.tensor_add(sc[:, 0:n], sc[:, 0:n],
                                                 mask_sb[r][:, off:SQT])
                        e = expp.tile([128, SQT], bf16, tag="e",
                                      name=f"e{sqT}_{h}_{i}")
                        nc.scalar.activation(e[:, 0:n], sc[:, 0:n],
                                             mybir.ActivationFunctionType.Exp,
                                             scale=SCALE)
                        exp_tiles.append((e, off))
                        # denominator accumulation entirely on DVE so the ACT
                        # engine's exp queue is never delayed by an init copy:
                        # quarter 0 starts from a memset (block 1 is already
                        # diagonal there); later quarters pair-add the first
                        # two (full-width) exp tiles
                        if i == 0:
                            if sqT == 0:
                                nc.any.memset(eacc[:], 0.0)
                                nc.vector.tensor_add(eacc[:], eacc[:], e[:])
                        elif i == 1 and sqT > 0:
                            nc.vector.tensor_add(eacc[:], exp_tiles[0][0][:],
                                                 e[:])
                        else:
                            nc.vector.tensor_add(eacc[:, off:SQT],
                                                 eacc[:, off:SQT], e[:, 0:n])
                        if i >= PVLAG:
                            emit_pv(i - PVLAG)
                        blocks_done += 1
                        if op is not None:
                            span = half if oq_next is not None else nblk_total
                            quota = min(NK, NK * blocks_done // span)
                            while op.d < quota:
                                op.step()
                        elif op2 is not None:
                            quota = op2.na * (blocks_done - half) \
                                // (nblk_total - half)
                            while op2.d < min(quota, op2.na):
                                op2.step()
                    for j in range(nblk - PVLAG, nblk):
                        emit_pv(j)

                    # evacuate the PV accumulator to SBUF with a cheap ACT
                    # copy so its PSUM bank frees without riding the
                    # den->recip->broadcast DVE chain
                    aev = aevp.tile([HD, SQT], f32, tag="ae",
                                    name=f"aev{sqT}_{h}")
                    nc.scalar.copy(aev[:], attn_ps[:])
                    # den: one [128x1] ones matmul -> column sums; tiny
                    # reciprocal on DVE, partition broadcast on GpSimd
                    den_ps = ps.tile([1, SQT], f32, tag="b",
                                     name=f"dps{sqT}_{h}")
                    nc.tensor.matmul(den_ps[:], ones_sb[:, 0:1], eacc[:],
                                     start=True, stop=True)
                    rec = nrm.tile([1, SQT], f32, tag="rec",
                                   name=f"rec{sqT}_{h}")
                    nc.vector.reciprocal(rec[:], den_ps[:])
                    bc = nrm.tile([128, SQT], f32, tag="bc",
                                  name=f"bc{sqT}_{h}")
                    nc.gpsimd.partition_broadcast(bc[:], rec[:], channels=128)
                    a_sb = attnsb.tile([HD, SQT], bf16, tag="a",
                                       name=f"asb{sqT}_{h}")
                    nc.vector.tensor_mul(a_sb[:], aev[:], bc[:])

                    # ship this head's slice; chunked AllGather (quarter 3 is
                    # split 3+1 so the tail projection covers the B latency)
                    asplit = AG_SPLIT[sqT]
                    chunk = 0 if h < asplit else 1
                    row = (h if chunk == 0 else h - asplit) * HD
                    nc.sync.dma_start(ag_in[sqT][chunk][row:row + HD, :],
                                      a_sb[:])
                    if h == asplit - 1 or h == HPC - 1:
                        nc.gpsimd.collective_compute(
                            "AllGather", mybir.AluOpType.bypass,
                            replica_groups=rg,
                            ins=[ag_in[sqT][chunk].opt()],
                            outs=[ag_out[sqT][chunk].opt()])
                    # hand off to the next quarter's projection at the head
                    # boundary, where only the finished o_ps tiles are live
                    if (oq_next is not None and op is not None
                            and blocks_done >= half):
                        op.finish()
                        op = None
                        op2 = OProj(oq_next, a_first=True, defer_b=True)
                if op is not None:
                    op.finish()
                return op2

            # ---------------- schedule ----------------
            x0, w0 = emit_x_loads(0)
            emit_qkv(0, x0, w0)
            x1, w1 = emit_x_loads(1)   # drains during round 0 compute
            emit_qkv(1, x1, w1)
            x2, w2 = emit_x_loads(2)   # drains during attention 0
            for d in range(NK):    # wo needed first at oproj(0) in attn(1)
                nc.sync.dma_start(wo_sb[d][:], wo[d * 128:(d + 1) * 128, :])
            emit_attention(0, None)
            emit_qkv(2, x2, w2)
            x3, w3 = emit_x_loads(3)   # drains during attention 1
            emit_attention(1, 0)
            emit_qkv(3, x3, w3)
            emit_out_stores(0)
            emit_attention(2, 1)
            emit_out_stores(1)
            op3 = emit_attention(3, 2, oq_next=3)
            emit_out_stores(2)
            op3.finish()
            emit_out_stores(3)

    nc.compile()
    return nc


def _prep_inputs(x, wq, wk, wv, wo, freqs_cos, freqs_sin, mask):
    bf16 = ml_dtypes.bfloat16
    x2 = np.asarray(x, dtype=np.float32).reshape(S, D)
    xT = np.ascontiguousarray(x2.T).astype(bf16)
    cosT = np.repeat(np.asarray(freqs_cos, np.float32).T, 2, axis=0)
    sinT = np.repeat(np.asarray(freqs_sin, np.float32).T, 2, axis=0).copy()
    sinT[0::2] *= -1.0
    cosT = np.ascontiguousarray(cosT).astype(bf16)
    sinT = np.ascontiguousarray(sinT).astype(bf16)
    m2 = np.asarray(mask, np.float32).reshape(S, S)
    masks = np.stack([np.ascontiguousarray(m2[0:SQT, r * 128:(r + 1) * 128].T)
                      for r in range(4)]).astype(bf16)  # [4, 128, 512]
    in_maps = []
    for c in range(N_CORES):
        cols = slice(c * CW, (c + 1) * CW)
        in_maps.append({
            "xT": xT,
            "wq": np.ascontiguousarray(np.asarray(wq, np.float32)[:, cols]).astype(bf16),
            "wk": np.ascontiguousarray(np.asarray(wk, np.float32)[:, cols]).astype(bf16),
            "wv": np.ascontiguousarray(np.asarray(wv, np.float32)[:, cols]).astype(bf16),
            "wo": np.ascontiguousarray(np.asarray(wo, np.float32)[:, cols]).astype(bf16),
            "cosT": cosT,
            "ones": np.ones((128, 128), bf16),
            "sinT": sinT,
            "masks": masks,
        })
    return in_maps


def kernel(x, wq, wk, wv, wo, freqs_cos, freqs_sin, mask):
    global LAST_RESULT
    from concourse.bass_utils import run_bass_kernel_spmd

    if "nc" not in _CACHE:
        _CACHE["nc"] = _build()
    nc = _CACHE["nc"]
    in_maps = _prep_inputs(x, wq, wk, wv, wo, freqs_cos, freqs_sin, mask)
    res = run_bass_kernel_spmd(nc, in_maps, core_ids=list(range(N_CORES)))
    LAST_RESULT = res
    out = np.concatenate([res.results[c]["out"] for c in range(N_CORES)],
                         axis=1)
    return out.reshape(B, S, D).astype(np.float32)


# revision 48
# speedup vs baseline: 1.0011x; 1.0011x over previous
"""Trainium2 Bass kernel for a LLaMA-style causal attention block.

Sharding (8 NeuronCores, one trn2 chip):
  - Tensor-parallel over heads: core c owns heads [4c, 4c+4) -> wq/wk/wv column
    slices [4096, 512]; computes qT/kT/v + RoPE + causal attention for its heads.
  - attnT [512, 2048] (bf16) is AllGather'd per sq quarter in two head-pair
    chunks (issued mid-attention so comm overlaps compute) -> each core computes
    out[:, 512c:512c+512] = attn @ wo_cols, interleaved into the next quarter's
    attention so the PE never drains.
  - Host concatenates the 8 column slices.

Layout trick: everything is computed transposed ([head_dim, seq]) so that no
on-device transposes are needed anywhere:
  qT/kT = w_h.T @ xT      (xT host-pretransposed)
  scoresT[sk, sq] = kT_tile.T @ qT     (softmax denom off the PE critical path)
  attnT[hd, sq] = v_tile.T @ expT      (expT is exactly the scoresT layout)
  out[sq, cols] = attnT_full_tile.T @ wo_tile
RoPE is applied in the transposed layout with a DVE stream_shuffle partition
pair-swap. exp() needs no max-subtraction: scores are O(1) by construction.

Softmax denominator: exp tiles are accumulated on the DVE into an f32 SBUF
tile; one [128x128] all-ones matmul per (head, quarter) produces the
column-sums pre-broadcast across partitions (recip on DVE, no GpSimd).

x tile loads for round st+1 are emitted at the end of round st so the DMA
queues drain them during the attention phase instead of stalling the next
round's first matmuls.

Compute dtype bf16 (f32 PSUM accumulation), I/O f32.
"""

import math
import os
import sys

for _p in ("/opt/trn_rl_repo",):
    if os.path.isdir(_p) and _p not in sys.path:
        sys.path.insert(0, _p)

import numpy as np
import ml_dtypes

N_CORES = 8
B, S, D, H = 1, 2048, 4096, 32
HD = D // H          # 128
HPC = H // N_CORES   # 4 heads per core
CW = D // N_CORES    # 512 columns per core
NK = D // 128        # 32 contraction tiles
SQT = 512            # sq tile width
NSQ = S // SQT       # 4
SCALE = 1.0 / math.sqrt(HD)
PVLAG = 2            # sk-blocks of lead the exp pipeline gets over PV

_CACHE = {}
LAST_RESULT = None   # test harness reads exec_time_ns from here


def _build():
    import concourse.mybir as mybir
    import concourse.tile as tile
    from concourse import bacc, bass_isa

    dt = mybir.dt
    f32, bf16 = dt.float32, dt.bfloat16

    nc = bacc.Bacc("TRN2", target_bir_lowering=False, debug=False,
                   num_devices=N_CORES)

    xT = nc.dram_tensor("xT", [D, S], bf16, kind="ExternalInput").ap()
    wq = nc.dram_tensor("wq", [D, CW], bf16, kind="ExternalInput").ap()
    wk = nc.dram_tensor("wk", [D, CW], bf16, kind="ExternalInput").ap()
    wv = nc.dram_tensor("wv", [D, CW], bf16, kind="ExternalInput").ap()
    wo = nc.dram_tensor("wo", [D, CW], bf16, kind="ExternalInput").ap()
    cosT = nc.dram_tensor("cosT", [HD, S], bf16, kind="ExternalInput").ap()
    sinT = nc.dram_tensor("sinT", [HD, S], bf16, kind="ExternalInput").ap()
    ones = nc.dram_tensor("ones", [128, 128], bf16, kind="ExternalInput").ap()
    masks = nc.dram_tensor("masks", [4, 128, SQT], bf16, kind="ExternalInput").ap()
    out = nc.dram_tensor("out", [S, CW], f32, kind="ExternalOutput").ap()

    swap_mask = []
    for i in range(16):
        swap_mask += [2 * i + 1, 2 * i]

    rg = [list(range(N_CORES))]

    with tile.TileContext(nc) as tc:
        with (
            tc.tile_pool(name="consts", bufs=1) as cpool,
            tc.tile_pool(name="xp", bufs=34) as xpool,
            tc.tile_pool(name="wqp", bufs=10) as wqp,
            tc.tile_pool(name="wkp", bufs=10) as wkp,
            tc.tile_pool(name="wvp", bufs=11) as wvp,
            tc.tile_pool(name="res", bufs=1) as res,
            tc.tile_pool(name="qro", bufs=8) as qro,
            tc.tile_pool(name="rope32", bufs=4) as rope32,
            tc.tile_pool(name="ropebf", bufs=3) as ropebf,
            tc.tile_pool(name="expp", bufs=9) as expp,
            tc.tile_pool(name="accp", bufs=2) as accp,
            tc.tile_pool(name="aevp", bufs=2) as aevp,
            tc.tile_pool(name="nrm", bufs=2) as nrm,
            tc.tile_pool(name="attnsb", bufs=3) as attnsb,
            tc.tile_pool(name="wop", bufs=1) as wop,
            tc.tile_pool(name="agsb", bufs=6) as agsb,
            tc.tile_pool(name="osb", bufs=4) as osb,
            tc.tile_pool(name="ps", bufs=8, space="PSUM") as ps,
            tc.tile_pool(name="dram", bufs=1, space="DRAM") as dram,
        ):
            # resident results of K+rope and V; Q lives in a rotating pool
            krot = [res.tile([HD, S], bf16, name=f"krot{h}") for h in range(HPC)]
            v_sb = [res.tile([128, CW], bf16, name=f"v{i}") for i in range(S // 128)]

            # AllGather bounce buffers: per quarter, two head chunks (A, B);
            # quarter 3 is split 3+1 so the tail projection covers B's latency
            AG_SPLIT = [2, 2, 2, 2]
            ag_in = [[dram.tile([a * HD, SQT], bf16, name=f"agin{q}_{c}")
                      for c, a in enumerate((AG_SPLIT[q], HPC - AG_SPLIT[q]))]
                     for q in range(NSQ)]
            ag_out = [[dram.tile([N_CORES * a * HD, SQT], bf16,
                                 addr_space="Shared", name=f"agout{q}_{c}")
                       for c, a in enumerate((AG_SPLIT[q], HPC - AG_SPLIT[q]))]
                      for q in range(NSQ)]

            cos_sb = cpool.tile([HD, S], bf16, name="cos_sb")
            ones_sb = cpool.tile([128, 128], bf16, name="ones_sb")
            sin_sb = cpool.tile([HD, S], bf16, name="sin_sb")
            mask_sb = [cpool.tile([128, SQT], bf16, name=f"mask{r}")
                       for r in range(4)]
            wo_sb = [wop.tile([128, CW], bf16, name=f"wo{d}") for d in range(NK)]

            WPRE = 8   # weight tiles of the next round pre-emitted per proj
            WPRE0 = NK  # round 0 is fully cold: interleave every weight pair

            def emit_x_loads(st):
                # first 8 wq/wk tiles of round st go in FRONT of the x bulk so
                # the round's first matmuls never wait behind 4 MB of x
                sq0 = st * SQT
                pre_w = []
                tiles = []
                npre = WPRE0 if st == 0 else WPRE
                for d in range(NK):
                    if d < npre:
                        wqt = wqp.tile([128, CW], bf16, tag="wq",
                                       name=f"wq{st}_{d}")
                        nc.sync.dma_start(wqt[:], wq[d * 128:(d + 1) * 128, :])
                        wkt = wkp.tile([128, CW], bf16, tag="wk",
                                       name=f"wk{st}_{d}")
                        nc.sync.dma_start(wkt[:], wk[d * 128:(d + 1) * 128, :])
                        pre_w.append((wqt, wkt))
                    xt = xpool.tile([128, SQT], bf16, tag="x", name=f"x{st}_{d}")
                    nc.sync.dma_start(xt[:], xT[d * 128:(d + 1) * 128,
                                                sq0:sq0 + SQT])
                    tiles.append(xt)
                return tiles, pre_w

            qrot_tiles = {}  # (st, h) -> tile

            def emit_qkv(st, x_tiles, pre_w):
                sq0 = st * SQT
                q_ps = [ps.tile([128, SQT], f32, tag="b", name=f"qps{st}_{h}")
                        for h in range(HPC)]
                k_ps = [ps.tile([128, SQT], f32, tag="b", name=f"kps{st}_{h}")
                        for h in range(HPC)]
                for d in range(NK):
                    xt = x_tiles[d]
                    if d < len(pre_w):
                        wqt, wkt = pre_w[d]
                    else:
                        wqt = wqp.tile([128, CW], bf16, tag="wq",
                                       name=f"wq{st}_{d}")
                        nc.sync.dma_start(wqt[:], wq[d * 128:(d + 1) * 128, :])
                        wkt = wkp.tile([128, CW], bf16, tag="wk",
                                       name=f"wk{st}_{d}")
                        nc.sync.dma_start(wkt[:], wk[d * 128:(d + 1) * 128, :])
                    first, last = d == 0, d == NK - 1
                    for h in range(HPC):
                        nc.tensor.matmul(q_ps[h][:], wqt[:, h * HD:(h + 1) * HD],
                                         xt[:], start=first, stop=last)
                    for h in range(HPC):
                        nc.tensor.matmul(k_ps[h][:], wkt[:, h * HD:(h + 1) * HD],
                                         xt[:], start=first, stop=last)
                if st == 0:
                    # constants are first needed by RoPE / attention below;
                    # emitting them here keeps the first QKV DMAs in front
                    nc.sync.dma_start(cos_sb[:], cosT[:])
                    nc.sync.dma_start(sin_sb[:], sinT[:])
                    nc.sync.dma_start(ones_sb[:], ones[:])
                    for r in range(4):
                        nc.sync.dma_start(mask_sb[r][:], masks[r])
                # RoPE: rot = t*cos + shuffle(t)*sin'   (sin' sign-baked)
                for h in range(HPC):
                    qt = qro.tile([HD, SQT], bf16, tag="q", name=f"qrot{st}_{h}")
                    qrot_tiles[(st, h)] = qt
                    for pst, rot in ((q_ps[h], qt[:]),
                                     (k_ps[h], krot[h][:, sq0:sq0 + SQT])):
                        tbf = ropebf.tile([128, SQT], bf16, tag="rbf",
                                          name=f"rbf{st}_{h}")
                        nc.scalar.copy(tbf[:], pst[:])
                        tsw = ropebf.tile([128, SQT], bf16, tag="rsw",
                                          name=f"rsw{st}_{h}")
                        nc.vector.stream_shuffle(tsw[:], tbf[:], swap_mask)
                        t1 = rope32.tile([128, SQT], f32, tag="r32",
                                         name=f"r1_{st}_{h}")
                        nc.vector.tensor_mul(t1[:], tbf[:],
                                             cos_sb[:, sq0:sq0 + SQT])
                        t2 = rope32.tile([128, SQT], f32, tag="r32",
                                         name=f"r2_{st}_{h}")
                        nc.vector.tensor_mul(t2[:], tsw[:],
                                             sin_sb[:, sq0:sq0 + SQT])
                        nc.vector.tensor_add(rot, t1[:], t2[:])
                # V projection for this s range; all wv loads are issued
                # up front so the first V matmuls never wait on DMA
                wv_tiles = []
                for d in range(NK):
                    wvt = wvp.tile([128, CW], bf16, tag="wv", name=f"wv{st}_{d}")
                    nc.sync.dma_start(wvt[:], wv[d * 128:(d + 1) * 128, :])
                    wv_tiles.append(wvt)
                v_ps = [ps.tile([128, CW], f32, tag="b", name=f"vps{st}_{ss}")
                        for ss in range(4)]
                for d in range(NK):
                    first, last = d == 0, d == NK - 1
                    for ss in range(4):
                        nc.tensor.matmul(v_ps[ss][:],
                                         x_tiles[d][:, ss * 128:(ss + 1) * 128],
                                         wv_tiles[d][:], start=first, stop=last)
                for ss in range(4):
                    nc.scalar.copy(v_sb[st * 4 + ss][:], v_ps[ss][:])

            # ---------- output projection, interleaved per sk-block ----------
            pending_out = {}  # q -> o_tiles awaiting store

            class OProj:
                """Emits the output projection of quarter q in per-d slices so
                it can be spread across the next quarter's attention blocks,
                filling PE pipeline bubbles without spiking PSUM pressure."""

                def __init__(self, q, a_first=False):
                    self.q = q
                    self.d = 0
                    # a_first: consume the A head-chunk's d-tiles first so the
                    # tail projection's matmuls cover the B AllGather
                    sp = AG_SPLIT[q]
                    if a_first:
                        self.order = ([d for d in range(NK) if d % 4 < sp]
                                      + [d for d in range(NK) if d % 4 >= sp])
                    else:
                        self.order = list(range(NK))
                    self.agt = {}
                    self.o_ps = [ps.tile([128, CW], f32, tag="b",
                                         name=f"ops{q}_{ss}")
                                 for ss in range(4)]
                    self._load(0)
                    self._load(1)

                def _load(self, i):
                    if i >= NK:
                        return
                    d = self.order[i]
                    c, lh = d // 4, d % 4
                    sp = AG_SPLIT[self.q]
                    if lh < sp:
                        src, row = ag_out[self.q][0], c * sp * HD + lh * HD
                    else:
                        src = ag_out[self.q][1]
                        row = c * (HPC - sp) * HD + (lh - sp) * HD
                    agt = agsb.tile([128, SQT], bf16, tag="ag",
                                    name=f"agt{self.q}_{d}")
                    nc.sync.dma_start(agt[:], src[row:row + HD, :])
                    self.agt[d] = agt

                def step(self):
                    if self.d >= NK:
                        return
                    i = self.d
                    d = self.order[i]
                    self._load(i + 2)
                    first, last = i == 0, i == NK - 1
                    for ss in range(4):
                        nc.tensor.matmul(self.o_ps[ss][:],
                                         self.agt[d][:, ss * 128:(ss + 1) * 128],
                                         wo_sb[d][:], start=first, stop=last)
                    self.d += 1

                def finish(self):
                    while self.d < NK:
                        self.step()
                    o_tiles = []
                    for ss in range(4):
                        o = osb.tile([128, CW], f32, tag="o",
                                     name=f"o{self.q}_{ss}")
                        nc.scalar.copy(o[:], self.o_ps[ss][:])
                        o_tiles.append(o)
                    pending_out[self.q] = o_tiles

            def emit_out_stores(q):
                tiles = pending_out.pop(q)
                for ss in range(4):
                    nc.sync.dma_start(
                        out[q * SQT + ss * 128:q * SQT + (ss + 1) * 128, :],
                        tiles[ss][:])

            def emit_attention(sqT, oq, oq_next=None):
                # attention for quarter sqT; if oq is not None, interleave the
                # output projection of quarter oq between sk-blocks.  With
                # oq_next set (last quarter), oq is packed into the first half
                # and oq_next's A-chunk tiles into the second half so only its
                # B-chunk remains after the final AllGather.
                nblk = 4 * (sqT + 1)
                op = OProj(oq) if oq is not None else None
                op2 = None
                nblk_total = nblk * HPC
                half = nblk_total // 2
                na = NK // 2  # A-chunk d-tiles in a_first order
                blocks_done = 0
                for h in range(HPC):
                    attn_ps = ps.tile([HD, SQT], f32, tag="b",
                                      name=f"aps{sqT}_{h}")
                    eacc = accp.tile([128, SQT], bf16, tag="acc",
                                     name=f"acc{sqT}_{h}")
                    exp_tiles = []

                    def emit_pv(j, h=h, attn_ps=attn_ps,
                                exp_tiles=exp_tiles, nblk=nblk):
                        first, last = j == 0, j == nblk - 1
                        e, off = exp_tiles[j]
                        n = SQT - off
                        nc.tensor.matmul(attn_ps[:, off:SQT],
                                         v_sb[j][:, h * HD:(h + 1) * HD],
                                         e[:, 0:n],
                                         start=first, stop=last)

                    for i in range(nblk):
                        r = i - 4 * sqT
                        # diagonal blocks: only sq >= sk is valid; skip the
                        # fully-masked leading columns entirely
                        off = max(0, r) * 128
                        n = SQT - off
                        sc = ps.tile([128, SQT], f32, tag="b",
                                     name=f"sc{sqT}_{h}_{i}")
                        nc.tensor.matmul(sc[:, 0:n],
                                         krot[h][:, i * 128:(i + 1) * 128],
                                         qrot_tiles[(sqT, h)][:, off:SQT],
                                         start=True, stop=True)
                        if r >= 0:  # triangular part within the first strip
                            nc.vector.tensor_add(sc[:, 0:n], sc[:, 0:n],
                                                 mask_sb[r][:, off:SQT])
                        e = expp.tile([128, SQT], bf16, tag="e",
                                      name=f"e{sqT}_{h}_{i}")
                        nc.scalar.activation(e[:, 0:n], sc[:, 0:n],
                                             mybir.ActivationFunctionType.Exp,
                                             scale=SCALE)
                        exp_tiles.append((e, off))
                        # denominator accumulation on DVE, off the PE path
                        if i == 0:
                            nc.scalar.copy(eacc[:], e[:])
                        else:
                            nc.vector.tensor_add(eacc[:, off:SQT],
                                                 eacc[:, off:SQT], e[:, 0:n])
                        if i >= PVLAG:
                            emit_pv(i - PVLAG)
                        blocks_done += 1
                        if op is not None:
                            quota = NK * blocks_done // nblk_total
                            while op.d < quota:
                                op.step()
                    for j in range(nblk - PVLAG, nblk):
                        emit_pv(j)

                    # evacuate the PV accumulator to SBUF with a cheap ACT
                    # copy so its PSUM bank frees without riding the
                    # den->recip->broadcast DVE chain
                    aev = aevp.tile([HD, SQT], f32, tag="ae",
                                    name=f"aev{sqT}_{h}")
                    nc.scalar.copy(aev[:], attn_ps[:])
                    # den: one [128x1] ones matmul -> column sums; tiny
                    # reciprocal on DVE, partition broadcast on GpSimd
                    den_ps = ps.tile([1, SQT], f32, tag="b",
                                     name=f"dps{sqT}_{h}")
                    nc.tensor.matmul(den_ps[:], ones_sb[:, 0:1], eacc[:],
                                     start=True, stop=True)
                    rec = nrm.tile([1, SQT], f32, tag="rec",
                                   name=f"rec{sqT}_{h}")
                    nc.vector.reciprocal(rec[:], den_ps[:])
                    bc = nrm.tile([128, SQT], f32, tag="bc",
                                  name=f"bc{sqT}_{h}")
                    nc.gpsimd.partition_broadcast(bc[:], rec[:], channels=128)
                    a_sb = attnsb.tile([HD, SQT], bf16, tag="a",
                                       name=f"asb{sqT}_{h}")
                    nc.vector.tensor_mul(a_sb[:], aev[:], bc[:])

                    # ship this head's slice; chunked AllGather (quarter 3 is
                    # split 3+1 so the tail projection covers the B latency)
                    asplit = AG_SPLIT[sqT]
                    chunk = 0 if h < asplit else 1
                    row = (h if chunk == 0 else h - asplit) * HD
                    nc.sync.dma_start(ag_in[sqT][chunk][row:row + HD, :],
                                      a_sb[:])
                    if h == asplit - 1 or h == HPC - 1:
                        nc.gpsimd.collective_compute(
                            "AllGather", mybir.AluOpType.bypass,
                            replica_groups=rg,
                            ins=[ag_in[sqT][chunk].opt()],
                            outs=[ag_out[sqT][chunk].opt()])
                if op is not None:
                    op.finish()
                return op2

            # ---------------- schedule ----------------
            x0, w0 = emit_x_loads(0)
            emit_qkv(0, x0, w0)
            x1, w1 = emit_x_loads(1)   # drains during round 0 compute
            emit_qkv(1, x1, w1)
            x2, w2 = emit_x_loads(2)   # drains during attention 0
            for d in range(NK):    # wo needed first at oproj(0) in attn(1)
                nc.sync.dma_start(wo_sb[d][:], wo[d * 128:(d + 1) * 128, :])
            emit_attention(0, None)
            emit_qkv(2, x2, w2)
            x3, w3 = emit_x_loads(3)   # drains during attention 1
            emit_attention(1, 0)
            emit_qkv(3, x3, w3)
            emit_out_stores(0)
            emit_attention(2, 1)
            emit_out_stores(1)
            emit_attention(3, 2)
            emit_out_stores(2)
            op3 = OProj(3, a_first=True)
            op3.finish()
            emit_out_stores(3)

    nc.compile()
    return nc


def _prep_inputs(x, wq, wk, wv, wo, freqs_cos, freqs_sin, mask):
    bf16 = ml_dtypes.bfloat16
    x2 = np.asarray(x, dtype=np.float32).reshape(S, D)
    xT = np.ascontiguousarray(x2.T).astype(bf16)
    cosT = np.repeat(np.asarray(freqs_cos, np.float32).T, 2, axis=0)
    sinT = np.repeat(np.asarray(freqs_sin, np.float32).T, 2, axis=0).copy()
    sinT[0::2] *= -1.0
    cosT = np.ascontiguousarray(cosT).astype(bf16)
    sinT = np.ascontiguousarray(sinT).astype(bf16)
    m2 = np.asarray(mask, np.float32).reshape(S, S)
    masks = np.stack([np.ascontiguousarray(m2[0:SQT, r * 128:(r + 1) * 128].T)
                      for r in range(4)]).astype(bf16)  # [4, 128, 512]
    in_maps = []
    for c in range(N_CORES):
        cols = slice(c * CW, (c + 1) * CW)
        in_maps.append({
            "xT": xT,
            "wq": np.ascontiguousarray(np.asarray(wq, np.float32)[:, cols]).astype(bf16),
            "wk": np.ascontiguousarray(np.asarray(wk, np.float32)[:, cols]).astype(bf16),
            "wv": np.ascontiguousarray(np.asarray(wv, np.float32)[:, cols]).astype(bf16),
            "wo": np.ascontiguousarray(np.asarray(wo, np.float32)[:, cols]).astype(bf16),
            "cosT": cosT,
            "ones": np.ones((128, 128), bf16),
            "sinT": sinT,
            "masks": masks,
        })
    return in_maps


def kernel(x, wq, wk, wv, wo, freqs_cos, freqs_sin, mask):
    global LAST_RESULT
    from concourse.bass_utils import run_bass_kernel_spmd

    if "nc" not in _CACHE:
        _CACHE["nc"] = _build()
    nc = _CACHE["nc"]
    in_maps = _prep_inputs(x, wq, wk, wv, wo, freqs_cos, freqs_sin, mask)
    res = run_bass_kernel_spmd(nc, in_maps, core_ids=list(range(N_CORES)))
    LAST_RESULT = res
    out = np.concatenate([res.results[c]["out"] for c in range(N_CORES)],
                         axis=1)
    return out.reshape(B, S, D).astype(np.float32)


# revision 52
# speedup vs baseline: 1.0025x; 1.0013x over previous
"""Trainium2 Bass kernel for a LLaMA-style causal attention block.

Sharding (8 NeuronCores, one trn2 chip):
  - Tensor-parallel over heads: core c owns heads [4c, 4c+4) -> wq/wk/wv column
    slices [4096, 512]; computes qT/kT/v + RoPE + causal attention for its heads.
  - attnT [512, 2048] (bf16) is AllGather'd per sq quarter in two head-pair
    chunks (issued mid-attention so comm overlaps compute) -> each core computes
    out[:, 512c:512c+512] = attn @ wo_cols, interleaved into the next quarter's
    attention so the PE never drains.
  - Host concatenates the 8 column slices.

Layout trick: everything is computed transposed ([head_dim, seq]) so that no
on-device transposes are needed anywhere:
  qT/kT = w_h.T @ xT      (xT host-pretransposed)
  scoresT[sk, sq] = kT_tile.T @ qT     (softmax denom off the PE critical path)
  attnT[hd, sq] = v_tile.T @ expT      (expT is exactly the scoresT layout)
  out[sq, cols] = attnT_full_tile.T @ wo_tile
RoPE is applied in the transposed layout with a DVE stream_shuffle partition
pair-swap. exp() needs no max-subtraction: scores are O(1) by construction.

Softmax denominator: exp tiles are accumulated on the DVE into an f32 SBUF
tile; one [128x128] all-ones matmul per (head, quarter) produces the
column-sums pre-broadcast across partitions (recip on DVE, no GpSimd).

x tile loads for round st+1 are emitted at the end of round st so the DMA
queues drain them during the attention phase instead of stalling the next
round's first matmuls.

Compute dtype bf16 (f32 PSUM accumulation), I/O f32.
"""

import math
import os
import sys

for _p in ("/opt/trn_rl_repo",):
    if os.path.isdir(_p) and _p not in sys.path:
        sys.path.insert(0, _p)

import numpy as np
import ml_dtypes

N_CORES = 8
B, S, D, H = 1, 2048, 4096, 32
HD = D // H          # 128
HPC = H // N_CORES   # 4 heads per core
CW = D // N_CORES    # 512 columns per core
NK = D // 128        # 32 contraction tiles
SQT = 512            # sq tile width
NSQ = S // SQT       # 4
SCALE = 1.0 / math.sqrt(HD)
PVLAG = 2            # sk-blocks of lead the exp pipeline gets over PV

_CACHE = {}
LAST_RESULT = None   # test harness reads exec_time_ns from here


def _build():
    import concourse.mybir as mybir
    import concourse.tile as tile
    from concourse import bacc, bass_isa

    dt = mybir.dt
    f32, bf16 = dt.float32, dt.bfloat16

    nc = bacc.Bacc("TRN2", target_bir_lowering=False, debug=False,
                   num_devices=N_CORES)

    xT = nc.dram_tensor("xT", [D, S], bf16, kind="ExternalInput").ap()
    wq = nc.dram_tensor("wq", [D, CW], bf16, kind="ExternalInput").ap()
    wk = nc.dram_tensor("wk", [D, CW], bf16, kind="ExternalInput").ap()
    wv = nc.dram_tensor("wv", [D, CW], bf16, kind="ExternalInput").ap()
    wo = nc.dram_tensor("wo", [D, CW], bf16, kind="ExternalInput").ap()
    cosT = nc.dram_tensor("cosT", [HD, S], bf16, kind="ExternalInput").ap()
    sinT = nc.dram_tensor("sinT", [HD, S], bf16, kind="ExternalInput").ap()
    ones = nc.dram_tensor("ones", [128, 128], bf16, kind="ExternalInput").ap()
    masks = nc.dram_tensor("masks", [4, 128, SQT], bf16, kind="ExternalInput").ap()
    out = nc.dram_tensor("out", [S, CW], f32, kind="ExternalOutput").ap()

    swap_mask = []
    for i in range(16):
        swap_mask += [2 * i + 1, 2 * i]

    rg = [list(range(N_CORES))]

    with tile.TileContext(nc) as tc:
        with (
            tc.tile_pool(name="consts", bufs=1) as cpool,
            tc.tile_pool(name="xp", bufs=34) as xpool,
            tc.tile_pool(name="wqp", bufs=10) as wqp,
            tc.tile_pool(name="wkp", bufs=10) as wkp,
            tc.tile_pool(name="wvp", bufs=11) as wvp,
            tc.tile_pool(name="res", bufs=1) as res,
            tc.tile_pool(name="qro", bufs=8) as qro,
            tc.tile_pool(name="rope32", bufs=4) as rope32,
            tc.tile_pool(name="ropebf", bufs=3) as ropebf,
            tc.tile_pool(name="expp", bufs=9) as expp,
            tc.tile_pool(name="accp", bufs=2) as accp,
            tc.tile_pool(name="aevp", bufs=2) as aevp,
            tc.tile_pool(name="nrm", bufs=2) as nrm,
            tc.tile_pool(name="attnsb", bufs=3) as attnsb,
            tc.tile_pool(name="wop", bufs=1) as wop,
            tc.tile_pool(name="agsb", bufs=6) as agsb,
            tc.tile_pool(name="osb", bufs=4) as osb,
            tc.tile_pool(name="ps", bufs=8, space="PSUM") as ps,
            tc.tile_pool(name="dram", bufs=1, space="DRAM") as dram,
        ):
            # resident results of K+rope and V; Q lives in a rotating pool
            krot = [res.tile([HD, S], bf16, name=f"krot{h}") for h in range(HPC)]
            v_sb = [res.tile([128, CW], bf16, name=f"v{i}") for i in range(S // 128)]

            # AllGather bounce buffers: per quarter, two head chunks (A, B);
            # quarter 3 is split 3+1 so the tail projection covers B's latency
            AG_SPLIT = [2, 2, 2, 2]
            ag_in = [[dram.tile([a * HD, SQT], bf16, name=f"agin{q}_{c}")
                      for c, a in enumerate((AG_SPLIT[q], HPC - AG_SPLIT[q]))]
                     for q in range(NSQ)]
            ag_out = [[dram.tile([N_CORES * a * HD, SQT], bf16,
                                 addr_space="Shared", name=f"agout{q}_{c}")
                       for c, a in enumerate((AG_SPLIT[q], HPC - AG_SPLIT[q]))]
                      for q in range(NSQ)]

            cos_sb = cpool.tile([HD, S], bf16, name="cos_sb")
            ones_sb = cpool.tile([128, 128], bf16, name="ones_sb")
            sin_sb = cpool.tile([HD, S], bf16, name="sin_sb")
            mask_sb = [cpool.tile([128, SQT], bf16, name=f"mask{r}")
                       for r in range(4)]
            wo_sb = [wop.tile([128, CW], bf16, name=f"wo{d}") for d in range(NK)]

            WPRE = 8   # weight tiles of the next round pre-emitted per proj
            WPRE0 = NK  # round 0 is fully cold: interleave every weight pair

            def emit_x_loads(st):
                # first 8 wq/wk tiles of round st go in FRONT of the x bulk so
                # the round's first matmuls never wait behind 4 MB of x
                sq0 = st * SQT
                pre_w = []
                tiles = []
                npre = WPRE0 if st == 0 else WPRE
                for d in range(NK):
                    if d < npre:
                        wqt = wqp.tile([128, CW], bf16, tag="wq",
                                       name=f"wq{st}_{d}")
                        nc.sync.dma_start(wqt[:], wq[d * 128:(d + 1) * 128, :])
                        wkt = wkp.tile([128, CW], bf16, tag="wk",
                                       name=f"wk{st}_{d}")
                        nc.sync.dma_start(wkt[:], wk[d * 128:(d + 1) * 128, :])
                        pre_w.append((wqt, wkt))
                    xt = xpool.tile([128, SQT], bf16, tag="x", name=f"x{st}_{d}")
                    nc.sync.dma_start(xt[:], xT[d * 128:(d + 1) * 128,
                                                sq0:sq0 + SQT])
                    tiles.append(xt)
                return tiles, pre_w

            qrot_tiles = {}  # (st, h) -> tile

            def emit_qkv(st, x_tiles, pre_w, attn_gen=None):
                sq0 = st * SQT
                q_ps = [ps.tile([128, SQT], f32, tag="b", name=f"qps{st}_{h}")
                        for h in range(HPC)]
                k_ps = [ps.tile([128, SQT], f32, tag="b", name=f"kps{st}_{h}")
                        for h in range(HPC)]
                for d in range(NK):
                    xt = x_tiles[d]
                    if d < len(pre_w):
                        wqt, wkt = pre_w[d]
                    else:
                        wqt = wqp.tile([128, CW], bf16, tag="wq",
                                       name=f"wq{st}_{d}")
                        nc.sync.dma_start(wqt[:], wq[d * 128:(d + 1) * 128, :])
                        wkt = wkp.tile([128, CW], bf16, tag="wk",
                                       name=f"wk{st}_{d}")
                        nc.sync.dma_start(wkt[:], wk[d * 128:(d + 1) * 128, :])
                    first, last = d == 0, d == NK - 1
                    for h in range(HPC):
                        nc.tensor.matmul(q_ps[h][:], wqt[:, h * HD:(h + 1) * HD],
                                         xt[:], start=first, stop=last)
                    for h in range(HPC):
                        nc.tensor.matmul(k_ps[h][:], wkt[:, h * HD:(h + 1) * HD],
                                         xt[:], start=first, stop=last)
                if st == 0:
                    # constants are first needed by RoPE / attention below;
                    # emitting them here keeps the first QKV DMAs in front
                    nc.sync.dma_start(cos_sb[:], cosT[:])
                    nc.sync.dma_start(sin_sb[:], sinT[:])
                    nc.sync.dma_start(ones_sb[:], ones[:])
                    for r in range(4):
                        nc.sync.dma_start(mask_sb[r][:], masks[r])
                # RoPE: rot = t*cos + shuffle(t)*sin'   (sin' sign-baked)
                for h in range(HPC):
                    qt = qro.tile([HD, SQT], bf16, tag="q", name=f"qrot{st}_{h}")
                    qrot_tiles[(st, h)] = qt
                    for pst, rot in ((q_ps[h], qt[:]),
                                     (k_ps[h], krot[h][:, sq0:sq0 + SQT])):
                        tbf = ropebf.tile([128, SQT], bf16, tag="rbf",
                                          name=f"rbf{st}_{h}")
                        nc.scalar.copy(tbf[:], pst[:])
                        tsw = ropebf.tile([128, SQT], bf16, tag="rsw",
                                          name=f"rsw{st}_{h}")
                        nc.vector.stream_shuffle(tsw[:], tbf[:], swap_mask)
                        t1 = rope32.tile([128, SQT], f32, tag="r32",
                                         name=f"r1_{st}_{h}")
                        nc.vector.tensor_mul(t1[:], tbf[:],
                                             cos_sb[:, sq0:sq0 + SQT])
                        t2 = rope32.tile([128, SQT], f32, tag="r32",
                                         name=f"r2_{st}_{h}")
                        nc.vector.tensor_mul(t2[:], tsw[:],
                                             sin_sb[:, sq0:sq0 + SQT])
                        nc.vector.tensor_add(rot, t1[:], t2[:])
                # V projection for this s range; all wv loads are issued
                # up front so the first V matmuls never wait on DMA
                wv_tiles = []
                for d in range(NK):
                    wvt = wvp.tile([128, CW], bf16, tag="wv", name=f"wv{st}_{d}")
                    nc.sync.dma_start(wvt[:], wv[d * 128:(d + 1) * 128, :])
                    wv_tiles.append(wvt)
                v_ps = [ps.tile([128, CW], f32, tag="b", name=f"vps{st}_{ss}")
                        for ss in range(4)]
                for d in range(NK):
                    first, last = d == 0, d == NK - 1
                    for ss in range(4):
                        nc.tensor.matmul(v_ps[ss][:],
                                         x_tiles[d][:, ss * 128:(ss + 1) * 128],
                                         wv_tiles[d][:], start=first, stop=last)
                    # weave the previous quarter's attention blocks between
                    # v-d-steps: its exp (ACT) waits hide under v matmuls
                    if attn_gen is not None and d % 2 == 1:
                        next(attn_gen, None)
                for ss in range(4):
                    nc.scalar.copy(v_sb[st * 4 + ss][:], v_ps[ss][:])
                if attn_gen is not None:
                    for _ in attn_gen:
                        pass

            # ---------- output projection, interleaved per sk-block ----------
            pending_out = {}  # q -> o_tiles awaiting store

            class OProj:
                """Emits the output projection of quarter q in per-d slices so
                it can be spread across the next quarter's attention blocks,
                filling PE pipeline bubbles without spiking PSUM pressure."""

                def __init__(self, q, a_first=False):
                    self.q = q
                    self.d = 0
                    # a_first: consume the A head-chunk's d-tiles first so the
                    # tail projection's matmuls cover the B AllGather
                    sp = AG_SPLIT[q]
                    if a_first:
                        self.order = ([d for d in range(NK) if d % 4 < sp]
                                      + [d for d in range(NK) if d % 4 >= sp])
                    else:
                        self.order = list(range(NK))
                    self.agt = {}
                    self.o_ps = [ps.tile([128, CW], f32, tag="b",
                                         name=f"ops{q}_{ss}")
                                 for ss in range(4)]
                    self._load(0)
                    self._load(1)

                def _load(self, i):
                    if i >= NK:
                        return
                    d = self.order[i]
                    c, lh = d // 4, d % 4
                    sp = AG_SPLIT[self.q]
                    if lh < sp:
                        src, row = ag_out[self.q][0], c * sp * HD + lh * HD
                    else:
                        src = ag_out[self.q][1]
                        row = c * (HPC - sp) * HD + (lh - sp) * HD
                    agt = agsb.tile([128, SQT], bf16, tag="ag",
                                    name=f"agt{self.q}_{d}")
                    nc.sync.dma_start(agt[:], src[row:row + HD, :])
                    self.agt[d] = agt

                def step(self):
                    if self.d >= NK:
                        return
                    i = self.d
                    d = self.order[i]
                    self._load(i + 2)
                    first, last = i == 0, i == NK - 1
                    for ss in range(4):
                        nc.tensor.matmul(self.o_ps[ss][:],
                                         self.agt[d][:, ss * 128:(ss + 1) * 128],
                                         wo_sb[d][:], start=first, stop=last)
                    self.d += 1

                def finish(self):
                    while self.d < NK:
                        self.step()
                    o_tiles = []
                    for ss in range(4):
                        o = osb.tile([128, CW], f32, tag="o",
                                     name=f"o{self.q}_{ss}")
                        nc.scalar.copy(o[:], self.o_ps[ss][:])
                        o_tiles.append(o)
                    pending_out[self.q] = o_tiles

            def emit_out_stores(q):
                tiles = pending_out.pop(q)
                for ss in range(4):
                    nc.sync.dma_start(
                        out[q * SQT + ss * 128:q * SQT + (ss + 1) * 128, :],
                        tiles[ss][:])

            def emit_attention(sqT, oq, oq_next=None):
                # attention for quarter sqT; if oq is not None, interleave the
                # output projection of quarter oq between sk-blocks.  With
                # oq_next set (last quarter), oq is packed into the first half
                # and oq_next's A-chunk tiles into the second half so only its
                # B-chunk remains after the final AllGather.
                nblk = 4 * (sqT + 1)
                op = OProj(oq) if oq is not None else None
                op2 = None
                nblk_total = nblk * HPC
                half = nblk_total // 2
                na = NK // 2  # A-chunk d-tiles in a_first order
                blocks_done = 0
                for h in range(HPC):
                    attn_ps = ps.tile([HD, SQT], f32, tag="b",
                                      name=f"aps{sqT}_{h}")
                    eacc = accp.tile([128, SQT], bf16, tag="acc",
                                     name=f"acc{sqT}_{h}")
                    exp_tiles = []

                    def emit_pv(j, h=h, attn_ps=attn_ps,
                                exp_tiles=exp_tiles, nblk=nblk):
                        first, last = j == 0, j == nblk - 1
                        e, off = exp_tiles[j]
                        n = SQT - off
                        nc.tensor.matmul(attn_ps[:, off:SQT],
                                         v_sb[j][:, h * HD:(h + 1) * HD],
                                         e[:, 0:n],
                                         start=first, stop=last)

                    for i in range(nblk):
                        r = i - 4 * sqT
                        # diagonal blocks: only sq >= sk is valid; skip the
                        # fully-masked leading columns entirely
                        off = max(0, r) * 128
                        n = SQT - off
                        sc = ps.tile([128, SQT], f32, tag="b",
                                     name=f"sc{sqT}_{h}_{i}")
                        nc.tensor.matmul(sc[:, 0:n],
                                         krot[h][:, i * 128:(i + 1) * 128],
                                         qrot_tiles[(sqT, h)][:, off:SQT],
                                         start=True, stop=True)
                        if r >= 0:  # triangular part within the first strip
                            nc.vector.tensor_add(sc[:, 0:n], sc[:, 0:n],
                                                 mask_sb[r][:, off:SQT])
                        e = expp.tile([128, SQT], bf16, tag="e",
                                      name=f"e{sqT}_{h}_{i}")
                        nc.scalar.activation(e[:, 0:n], sc[:, 0:n],
                                             mybir.ActivationFunctionType.Exp,
                                             scale=SCALE)
                        exp_tiles.append((e, off))
                        # denominator accumulation on DVE, off the PE path
                        if i == 0:
                            nc.scalar.copy(eacc[:], e[:])
                        else:
                            nc.vector.tensor_add(eacc[:, off:SQT],
                                                 eacc[:, off:SQT], e[:, 0:n])
                        if i >= PVLAG:
                            emit_pv(i - PVLAG)
                        blocks_done += 1
                        if op is not None:
                            quota = NK * blocks_done // nblk_total
                            while op.d < quota:
                                op.step()
                    for j in range(nblk - PVLAG, nblk):
                        emit_pv(j)

                    # evacuate the PV accumulator to SBUF with a cheap ACT
                    # copy so its PSUM bank frees without riding the
                    # den->recip->broadcast DVE chain
                    aev = aevp.tile([HD, SQT], f32, tag="ae",
                                    name=f"aev{sqT}_{h}")
                    nc.scalar.copy(aev[:], attn_ps[:])
                    # den: one [128x1] ones matmul -> column sums; tiny
                    # reciprocal on DVE, partition broadcast on GpSimd
                    den_ps = ps.tile([1, SQT], f32, tag="b",
                                     name=f"dps{sqT}_{h}")
                    nc.tensor.matmul(den_ps[:], ones_sb[:, 0:1], eacc[:],
                                     start=True, stop=True)
                    rec = nrm.tile([1, SQT], f32, tag="rec",
                                   name=f"rec{sqT}_{h}")
                    nc.vector.reciprocal(rec[:], den_ps[:])
                    bc = nrm.tile([128, SQT], f32, tag="bc",
                                  name=f"bc{sqT}_{h}")
                    nc.gpsimd.partition_broadcast(bc[:], rec[:], channels=128)
                    a_sb = attnsb.tile([HD, SQT], bf16, tag="a",
                                       name=f"asb{sqT}_{h}")
                    nc.vector.tensor_mul(a_sb[:], aev[:], bc[:])

                    # ship this head's slice; chunked AllGather (quarter 3 is
                    # split 3+1 so the tail projection covers the B latency)
                    asplit = AG_SPLIT[sqT]
                    chunk = 0 if h < asplit else 1
                    row = (h if chunk == 0 else h - asplit) * HD
                    nc.sync.dma_start(ag_in[sqT][chunk][row:row + HD, :],
                                      a_sb[:])
                    if h == asplit - 1 or h == HPC - 1:
                        nc.gpsimd.collective_compute(
                            "AllGather", mybir.AluOpType.bypass,
                            replica_groups=rg,
                            ins=[ag_in[sqT][chunk].opt()],
                            outs=[ag_out[sqT][chunk].opt()])
                if op is not None:
                    op.finish()
                return op2

            def gen_attention0():
                # quarter-0 attention as a generator: one sk-block per resume,
                # woven into qkv(1)'s V-loop so exp waits hide under v matmuls
                sqT, nblk = 0, 4
                for h in range(HPC):
                    attn_ps = ps.tile([HD, SQT], f32, tag="b",
                                      name=f"aps0_{h}")
                    eacc = accp.tile([128, SQT], bf16, tag="acc",
                                     name=f"acc0_{h}")
                    exp_tiles = []

                    def emit_pv(j, h=h, attn_ps=attn_ps,
                                exp_tiles=exp_tiles, nblk=nblk):
                        first, last = j == 0, j == nblk - 1
                        e, off = exp_tiles[j]
                        n = SQT - off
                        nc.tensor.matmul(attn_ps[:, off:SQT],
                                         v_sb[j][:, h * HD:(h + 1) * HD],
                                         e[:, 0:n],
                                         start=first, stop=last)

                    for i in range(nblk):
                        off = i * 128
                        n = SQT - off
                        sc = ps.tile([128, SQT], f32, tag="b",
                                     name=f"sc0_{h}_{i}")
                        nc.tensor.matmul(sc[:, 0:n],
                                         krot[h][:, i * 128:(i + 1) * 128],
                                         qrot_tiles[(0, h)][:, off:SQT],
                                         start=True, stop=True)
                        nc.vector.tensor_add(sc[:, 0:n], sc[:, 0:n],
                                             mask_sb[i][:, off:SQT])
                        e = expp.tile([128, SQT], bf16, tag="e",
                                      name=f"e0_{h}_{i}")
                        nc.scalar.activation(e[:, 0:n], sc[:, 0:n],
                                             mybir.ActivationFunctionType.Exp,
                                             scale=SCALE)
                        exp_tiles.append((e, off))
                        if i == 0:
                            nc.scalar.copy(eacc[:], e[:])
                        else:
                            nc.vector.tensor_add(eacc[:, off:SQT],
                                                 eacc[:, off:SQT], e[:, 0:n])
                        if i >= PVLAG:
                            emit_pv(i - PVLAG)
                        yield
                    for j in range(nblk - PVLAG, nblk):
                        emit_pv(j)
                    aev = aevp.tile([HD, SQT], f32, tag="ae", name=f"aev0_{h}")
                    nc.scalar.copy(aev[:], attn_ps[:])
                    den_ps = ps.tile([1, SQT], f32, tag="b", name=f"dps0_{h}")
                    nc.tensor.matmul(den_ps[:], ones_sb[:, 0:1], eacc[:],
                                     start=True, stop=True)
                    rec = nrm.tile([1, SQT], f32, tag="rec", name=f"rec0_{h}")
                    nc.vector.reciprocal(rec[:], den_ps[:])
                    bc = nrm.tile([128, SQT], f32, tag="bc", name=f"bc0_{h}")
                    nc.gpsimd.partition_broadcast(bc[:], rec[:], channels=128)
                    a_sb = attnsb.tile([HD, SQT], bf16, tag="a",
                                       name=f"asb0_{h}")
                    nc.vector.tensor_mul(a_sb[:], aev[:], bc[:])
                    chunk = h // 2
                    nc.sync.dma_start(
                        ag_in[0][chunk][(h % 2) * HD:(h % 2 + 1) * HD, :],
                        a_sb[:])
                    if h % 2 == 1:
                        nc.gpsimd.collective_compute(
                            "AllGather", mybir.AluOpType.bypass,
                            replica_groups=rg,
                            ins=[ag_in[0][chunk].opt()],
                            outs=[ag_out[0][chunk].opt()])

            # ---------------- schedule ----------------
            x0, w0 = emit_x_loads(0)
            emit_qkv(0, x0, w0)
            x1, w1 = emit_x_loads(1)   # drains during round 0 compute
            emit_qkv(1, x1, w1, attn_gen=gen_attention0())
            x2, w2 = emit_x_loads(2)   # drains during round 1 tail
            for d in range(NK):    # wo needed first at oproj(0) in attn(1)
                nc.sync.dma_start(wo_sb[d][:], wo[d * 128:(d + 1) * 128, :])
            emit_qkv(2, x2, w2)
            x3, w3 = emit_x_loads(3)   # drains during attention 1
            emit_attention(1, 0)
            emit_qkv(3, x3, w3)
            emit_out_stores(0)
            emit_attention(2, 1)
            emit_out_stores(1)
            emit_attention(3, 2)
            emit_out_stores(2)
            op3 = OProj(3, a_first=True)
            op3.finish()
            emit_out_stores(3)

    nc.compile()
    return nc


def _prep_inputs(x, wq, wk, wv, wo, freqs_cos, freqs_sin, mask):
    bf16 = ml_dtypes.bfloat16
    x2 = np.asarray(x, dtype=np.float32).reshape(S, D)
    xT = np.ascontiguousarray(x2.T).astype(bf16)
    cosT = np.repeat(np.asarray(freqs_cos, np.float32).T, 2, axis=0)
    sinT = np.repeat(np.asarray(freqs_sin, np.float32).T, 2, axis=0).copy()
    sinT[0::2] *= -1.0
    cosT = np.ascontiguousarray(cosT).astype(bf16)
    sinT = np.ascontiguousarray(sinT).astype(bf16)
    m2 = np.asarray(mask, np.float32).reshape(S, S)
    masks = np.stack([np.ascontiguousarray(m2[0:SQT, r * 128:(r + 1) * 128].T)
                      for r in range(4)]).astype(bf16)  # [4, 128, 512]
    in_maps = []
    for c in range(N_CORES):
        cols = slice(c * CW, (c + 1) * CW)
        in_maps.append({
            "xT": xT,
            "wq": np.ascontiguousarray(np.asarray(wq, np.float32)[:, cols]).astype(bf16),
            "wk": np.ascontiguousarray(np.asarray(wk, np.float32)[:, cols]).astype(bf16),
            "wv": np.ascontiguousarray(np.asarray(wv, np.float32)[:, cols]).astype(bf16),
            "wo": np.ascontiguousarray(np.asarray(wo, np.float32)[:, cols]).astype(bf16),
            "cosT": cosT,
            "ones": np.ones((128, 128), bf16),
            "sinT": sinT,
            "masks": masks,
        })
    return in_maps


def kernel(x, wq, wk, wv, wo, freqs_cos, freqs_sin, mask):
    global LAST_RESULT
    from concourse.bass_utils import run_bass_kernel_spmd

    if "nc" not in _CACHE:
        _CACHE["nc"] = _build()
    nc = _CACHE["nc"]
    in_maps = _prep_inputs(x, wq, wk, wv, wo, freqs_cos, freqs_sin, mask)
    res = run_bass_kernel_spmd(nc, in_maps, core_ids=list(range(N_CORES)))
    LAST_RESULT = res
    out = np.concatenate([res.results[c]["out"] for c in range(N_CORES)],
                         axis=1)
    return out.reshape(B, S, D).astype(np.float32)
